# revision 36
# baseline (speedup 1.0000x reference)
"""Trainium2 Bass kernel for nn_DisentangleGraph (topk_masking).

Computes, for hidden (20000,256), H (20000,4096), int_emb (8,256):
  sim   = 10 * cosine(hidden, int_emb)                  (20000, 8)
  int_H = 2.0 where rank-within-column(sim) <= 6000     (top-6001 per column)
  H_out = concat([int_H, H], axis=1)                    (20000, 4104)
  degV  = rowsum(H_out);  degE = colmean of degV over nonzeros
  returns (H_out, degV**-0.5, degE**-0.5)

Sharding: node dimension split across 8 NeuronCores (2500 nodes each).
The per-column top-k threshold (the 6001-th largest sim value) is found with
a bisection over the value space after an AllGather of the (tiny) sim matrix;
degE needs an AllReduce of per-shard column sums.  Selection uses
`sim >= t` where t is the exact 6001-th largest value — equivalent to the
reference's double-argsort rank test whenever the threshold value is unique
in its column (holds for this input; verified against the reference).

Structure: the H stream (82 MB of DMA per core) must never wait for the
threshold search, so each H tile is copied to bf16 (exact for 0/1 data,
fused with the row-sum on the scalar engine) and written straight back out
to H_out[:, 8:].  The 8 int columns are staged and written once at the end;
a tiny read of H_out that overlaps both regions forces the int write after
every H write (odd rows share a 64-byte HBM line between the two, so they
must not overlap in time; Tile orders DMAs via DRAM range dependencies).
The column-sum matmuls run in bf16 (4x the fp32 PE rate) on the exact split
degV = 32*hi + lo; products and per-core partial sums stay below 2^24 so
nothing is rounded.
"""

import numpy as np

import concourse.bacc as bacc
import concourse.mybir as mybir
from concourse import bass_utils
from concourse.tile import TileContext

F32 = mybir.dt.float32
BF16 = mybir.dt.bfloat16
ALU = mybir.AluOpType
ACTF = mybir.ActivationFunctionType
AX = mybir.AxisListType

N_NODES = 20000
NUM_EDGE = 4096
DIM = 256
K_FAC = 8
NC = 8
SHARD = N_NODES // NC          # 2500
NT = (SHARD + 127) // 128      # 20 tiles per shard
SEL_CNT = 6001.0               # rank <= floor(0.3*N) selects 6001 values
TEMP = 10.0
EPS = 1e-8
N_ITER = 35                    # bisection iterations (converges to exact fp32)
WIDTH = NUM_EDGE + K_FAC       # 4104
T_BUFS = 3
TB_BUFS = 8

_CACHED = None


def _rows(i):
    return min(128, SHARD - i * 128)


def _build():
    nc = bacc.Bacc("TRN2", target_bir_lowering=False, debug=False, num_devices=NC)

    hid_d = nc.dram_tensor("hidden", [SHARD, DIM], F32, kind="ExternalInput")
    h_d = nc.dram_tensor("H", [SHARD, NUM_EDGE], F32, kind="ExternalInput")
    emb_d = nc.dram_tensor("int_emb", [K_FAC, DIM], F32, kind="ExternalInput")
    ident_d = nc.dram_tensor("ident", [128, 128], F32, kind="ExternalInput")
    g_d = nc.dram_tensor("G", [128, 128], F32, kind="ExternalInput")

    hout_d = nc.dram_tensor("Hout", [SHARD, WIDTH], F32, kind="ExternalOutput")
    degv_d = nc.dram_tensor("degV", [SHARD, 1], F32, kind="ExternalOutput")
    dege_d = nc.dram_tensor("degE", [WIDTH, 1], F32, kind="ExternalOutput")

    with TileContext(nc) as tc:
        with (
            tc.tile_pool(name="const", bufs=1) as cpool,
            tc.tile_pool(name="hid", bufs=3) as hpool,
            tc.tile_pool(name="hnt", bufs=3) as tpool_hnt,
            tc.tile_pool(name="bigT", bufs=T_BUFS) as Tpool,
            tc.tile_pool(name="bft", bufs=TB_BUFS) as Bpool,
            tc.tile_pool(name="small", bufs=3) as spool,
            tc.tile_pool(name="psA", bufs=1, space="PSUM") as psA,
            tc.tile_pool(name="psB", bufs=2, space="PSUM") as psB,
            tc.tile_pool(name="psC", bufs=2, space="PSUM") as psC,
            tc.tile_pool(name="dram", bufs=1, space="DRAM") as dpool,
        ):
            # ---------------- constants ----------------
            ident = cpool.tile([128, 128], F32)
            nc.scalar.dma_start(ident[:], ident_d[:])
            gmat = cpool.tile([128, 128], F32)
            nc.scalar.dma_start(gmat[:], g_d[:])
            ones_row = cpool.tile([1, 128], F32)
            nc.vector.memset(ones_row[:], 1.0)

            # ---------------- normalized int_emb, transposed ----------------
            emb = cpool.tile([K_FAC, DIM], F32)
            nc.scalar.dma_start(emb[:], emb_d[:])
            esq = cpool.tile([K_FAC, DIM], F32)
            ess = cpool.tile([K_FAC, 1], F32)
            nc.vector.tensor_tensor(esq[:], emb[:], emb[:], ALU.mult)
            nc.vector.reduce_sum(ess[:], esq[:], axis=AX.X)
            enorm = cpool.tile([K_FAC, 1], F32)
            nc.scalar.sqrt(enorm[:], ess[:])
            nc.vector.tensor_scalar_max(enorm[:], enorm[:], EPS)
            erin = cpool.tile([K_FAC, 1], F32)
            nc.vector.reciprocal(erin[:], enorm[:])
            en = cpool.tile([K_FAC, DIM], F32)
            nc.vector.tensor_scalar_mul(en[:], emb[:], erin[:])
            enT = cpool.tile([128, 2 * K_FAC], F32)  # chunk c at [:, c*8:(c+1)*8]
            for c in range(2):
                pt = psB.tile([128, 128], F32, tag="tr")
                nc.tensor.transpose(
                    pt[:128, :K_FAC], en[:, c * 128 : (c + 1) * 128],
                    ident[:K_FAC, :K_FAC],
                )
                nc.vector.tensor_copy(enT[:, c * K_FAC : (c + 1) * K_FAC],
                                      pt[:128, :K_FAC])

            # ---------------- phase A: local sim + transposed sim ----------------
            # (kept off the scalar engine: phase B's bf16 converts own it)
            # Normalization is batched: all 20 hidden tiles land in one
            # buffer and the square/rowsum/sqrt/reciprocal run as single
            # big ops instead of 20 short latency chains.
            sim_loc = cpool.tile([128, NT, K_FAC], F32)   # node-major local sim
            simT_loc = cpool.tile([K_FAC, SHARD], F32)    # column-major local sim
            hidall = cpool.tile([128, NT, DIM], F32)
            nc.vector.memset(hidall[:, NT - 1, :], 0.0)
            fullh = (NT - 1) * 128
            nc.scalar.dma_start(
                hidall[:, : NT - 1, :],
                hid_d[:fullh, :].rearrange("(i p) d -> p i d", p=128),
            )
            nc.scalar.dma_start(
                hidall[: SHARD - fullh, NT - 1, :], hid_d[fullh:, :]
            )
            sq_all = cpool.tile([128, NT * DIM], F32)
            nc.vector.tensor_tensor(
                sq_all[:], hidall[:].rearrange("p i d -> p (i d)"),
                hidall[:].rearrange("p i d -> p (i d)"), ALU.mult,
            )
            ss_all = cpool.tile([128, NT], F32)
            nc.vector.reduce_sum(
                ss_all[:], sq_all[:].rearrange("p (i d) -> p i d", d=DIM),
                axis=AX.X,
            )
            nrm_all = cpool.tile([128, NT], F32)
            nc.scalar.sqrt(nrm_all[:], ss_all[:])
            nc.vector.tensor_scalar_max(nrm_all[:], nrm_all[:], EPS)
            rin_all = cpool.tile([128, NT], F32)
            nc.vector.reciprocal(rin_all[:], nrm_all[:])
            for i in range(NT):
                rows = _rows(i)
                r0 = i * 128
                hn = hpool.tile([128, DIM], F32, tag="hn")
                nc.vector.tensor_scalar_mul(hn[:rows, :], hidall[:rows, i, :],
                                            rin_all[:rows, i : i + 1])

                hnT = tpool_hnt.tile([128, 256], F32, tag="hnT")
                for c in range(2):
                    pt = psB.tile([128, 128], F32, tag="tr")
                    nc.tensor.transpose(
                        pt[:128, :rows],
                        hn[:rows, c * 128 : (c + 1) * 128],
                        ident[:rows, :rows],
                    )
                    nc.vector.tensor_copy(hnT[:, c * 128 : c * 128 + rows],
                                          pt[:128, :rows])
                psim = psC.tile([128, 128], F32, tag="mm")
                for c in range(2):
                    nc.tensor.matmul(
                        psim[:rows, :K_FAC],
                        hnT[:, c * 128 : c * 128 + rows],
                        enT[:, c * K_FAC : (c + 1) * K_FAC],
                        start=(c == 0),
                        stop=(c == 1),
                    )
                # sim = TEMP * (hn @ en.T)
                nc.vector.tensor_scalar_mul(sim_loc[:rows, i, :],
                                            psim[:rows, :K_FAC], TEMP)
                ptT = psC.tile([128, 128], F32, tag="mm")
                nc.tensor.transpose(
                    ptT[:K_FAC, :rows], sim_loc[:rows, i, :], ident[:rows, :rows]
                )
                nc.vector.tensor_copy(simT_loc[:, r0 : r0 + rows],
                                      ptT[:K_FAC, :rows])

            # ---------------- AllGather sim ----------------
            simT_d = dpool.tile([K_FAC, SHARD], F32)
            nc.gpsimd.dma_start(simT_d[:], simT_loc[:])
            simfull_d = dpool.tile([NC * K_FAC, SHARD], F32, addr_space="Shared")
            nc.gpsimd.collective_compute(
                "AllGather",
                ALU.bypass,
                replica_groups=[list(range(NC))],
                ins=[simT_d[:].opt()],
                outs=[simfull_d[:].opt()],
            )
            # grouped layout: partition p = (q, h), q = rank*8+f, h in {0,1}
            # -> column of partition p is (p//2) % 8; 16 partitions per column.
            sim_g = cpool.tile([128, SHARD // 2], F32)
            nc.scalar.dma_start(
                sim_g[:], simfull_d[:].rearrange("q (h i) -> (q h) i", h=2)
            )

            # ---------------- bisection for per-column threshold ----------------
            # state: interval [lo, lo + 2*hw); probe mid = lo + hw; on
            # count >= target: lo = mid; always hw /= 2.  lo converges to the
            # exact fp32 value of the 6001-th largest element per column.
            lo = cpool.tile([128, 1], F32)
            hw = cpool.tile([128, 1], F32)
            mid = cpool.tile([128, 1], F32)
            nc.vector.memset(lo[:], -10.5)
            nc.vector.memset(hw[:], 10.5)
            nc.vector.memset(mid[:], 0.0)
            cmp_buf = cpool.tile([128, SHARD // 2], BF16)
            for it in range(N_ITER):
                pcnt = spool.tile([128, 1], F32, tag="pcnt")
                nc.vector.tensor_scalar(
                    out=cmp_buf[:],
                    in0=sim_g[:],
                    scalar1=mid[:],
                    scalar2=None,
                    op0=ALU.is_ge,
                    op1=ALU.add,
                    accum_out=pcnt[:],
                )
                pc = psC.tile([128, 128], F32, tag="mm")
                nc.tensor.matmul(pc[:128, :1], gmat[:], pcnt[:], start=True,
                                 stop=True)
                cnt = spool.tile([128, 1], F32, tag="cnt")
                nc.vector.tensor_copy(cnt[:], pc[:128, 0:1])
                geqf = spool.tile([128, 1], F32, tag="geqf")
                nc.vector.tensor_scalar(
                    out=geqf[:], in0=cnt[:], scalar1=SEL_CNT, scalar2=None,
                    op0=ALU.is_ge,
                )
                d = spool.tile([128, 1], F32, tag="d")
                nc.vector.tensor_tensor(d[:], geqf[:], hw[:], ALU.mult)
                nc.vector.tensor_tensor(lo[:], lo[:], d[:], ALU.add)
                nc.vector.tensor_scalar_mul(hw[:], hw[:], 0.5)
                nc.vector.tensor_tensor(mid[:], lo[:], hw[:], ALU.add)

            # threshold row (1, 8): column f lives (a.o.) on partition 2f
            th_row = cpool.tile([1, K_FAC], F32)
            nc.scalar.dma_start(th_row[:], lo[0:16:2, :])
            pbc = psC.tile([128, 128], F32, tag="mm")
            nc.tensor.matmul(pbc[:128, :K_FAC], ones_row[:], th_row[:],
                             start=True, stop=True)
            thr = cpool.tile([128, K_FAC], F32)
            nc.vector.tensor_copy(thr[:], pbc[:128, :K_FAC])

            # ---------------- phase B: stream H, build H_out, accumulate sums ----
            # Column-sum accumulators: chunk c in {0..8} (8 H chunks of 512
            # cols + the 8 int cols) accumulates rows [colsum(mask),
            # colsum(mask*hi), colsum(mask*lo)] as a (3, 512) PSUM region,
            # where degV = 32*hi + lo.  PE matmul outputs must start at
            # partition 0/32/64: chunk c -> bank paccs[c // 3], offset
            # 32 * (c % 3).
            paccs = [
                psA.tile([128, 512], F32, tag=f"pacc{t}", name=f"pacc{t}")
                for t in range(3)
            ]

            def acc_slice(c, width=512):
                return paccs[c // 3][32 * (c % 3) : 32 * (c % 3) + 3, :width]

            degv_all = cpool.tile([128, NT], F32)
            intall = cpool.tile([128, NT, K_FAC], F32)
            for i in range(NT):
                rows = _rows(i)
                r0 = i * 128
                T = Tpool.tile([128, NUM_EDGE], F32, tag="T")
                nc.sync.dma_start(T[:rows, :], h_d[r0 : r0 + rows, :])
                # H part of H_out goes out immediately — it does not depend
                # on the threshold search.
                nc.scalar.dma_start(hout_d[r0 : r0 + rows, K_FAC:], T[:rows, :])
                # f32 -> bf16 copy of H (exact: values are 0/1), fused with
                # the row-sum accumulation, on the scalar engine
                Tb = Bpool.tile([128, NUM_EDGE], BF16, tag="Tb")
                par = spool.tile([128, 8], F32, tag="par")
                for c in range(8):
                    nc.scalar.activation(
                        Tb[:rows, 512 * c : 512 * (c + 1)],
                        T[:rows, 512 * c : 512 * (c + 1)],
                        ACTF.Copy,
                        accum_out=par[:rows, c : c + 1],
                    )
                r = spool.tile([128, 1], F32, tag="r")
                nc.vector.reduce_sum(r[:rows, :], par[:rows, :], axis=AX.X)
                sel = spool.tile([128, K_FAC], F32, tag="sel")
                nc.vector.tensor_tensor(
                    sel[:rows, :], sim_loc[:rows, i, :], thr[:rows, :], ALU.is_ge
                )
                s2 = spool.tile([128, 1], F32, tag="s2")
                nc.scalar.activation(
                    intall[:rows, i, :], sel[:rows, :], ACTF.Copy, scale=2.0,
                    accum_out=s2[:rows, :],
                )
                Tbi = spool.tile([128, K_FAC], BF16, tag="Tbi")
                nc.scalar.activation(Tbi[:rows, :], sel[:rows, :], ACTF.Copy,
                                     scale=2.0)
                dv = spool.tile([128, 1], F32, tag="dv")
                nc.vector.tensor_tensor(dv[:rows, :], r[:rows, :], s2[:rows, :],
                                        ALU.add)
                # degV = 32*hi + lo, both bf16-exact (hi <= 128, lo <= 31).
                # hi = floor(dv/32) via round-to-nearest(dv/32 - 0.499): dv is
                # an exact integer so dv/32 sits on a 1/32 grid and the 0.499
                # offset keeps every grid point strictly inside its rounding
                # interval.
                t2 = spool.tile([128, 1], F32, tag="t2")
                nc.vector.tensor_scalar(
                    out=t2[:rows, :], in0=dv[:rows, :], scalar1=1.0 / 32.0,
                    scalar2=-0.499, op0=ALU.mult, op1=ALU.add,
                )
                hi_i = spool.tile([128, 1], mybir.dt.int32, tag="hi_i")
                nc.vector.tensor_copy(hi_i[:rows, :], t2[:rows, :])
                hiv = spool.tile([128, 1], F32, tag="hiv")
                nc.vector.tensor_copy(hiv[:rows, :], hi_i[:rows, :])
                lov = spool.tile([128, 1], F32, tag="lov")
                nc.vector.scalar_tensor_tensor(
                    lov[:rows, :], hiv[:rows, :], -32.0, dv[:rows, :],
                    ALU.mult, ALU.add,
                )
                lw = spool.tile([128, 3], BF16, tag="lw")
                nc.vector.memset(lw[:rows, 0:1], 1.0)
                nc.scalar.activation(lw[:rows, 1:2], hiv[:rows, :], ACTF.Copy)
                nc.scalar.activation(lw[:rows, 2:3], lov[:rows, :], ACTF.Copy)
                for c in range(8):
                    nc.tensor.matmul(
                        acc_slice(c),
                        lw[:rows, :],
                        Tb[:rows, 512 * c : 512 * (c + 1)],
                        start=(i == 0),
                        stop=(i == NT - 1),
                    )
                nc.tensor.matmul(
                    acc_slice(8, K_FAC),
                    lw[:rows, :],
                    Tbi[:rows, :],
                    start=(i == 0),
                    stop=(i == NT - 1),
                )
                rec = spool.tile([128, 1], F32, tag="rec")
                nc.vector.reciprocal(rec[:rows, :], dv[:rows, :])
                nc.scalar.sqrt(degv_all[:rows, i : i + 1], rec[:rows, :])

            # ---- int columns of H_out, written once, after every H write ----
            # Odd rows share a 64-byte HBM line with the H-part write of the
            # previous row, so the int write must not overlap any H write in
            # time.  The guard read overlaps every tile's H region (RAW: it
            # waits for all H writes) and the int region (WAR: the int write
            # waits for it).
            guard = cpool.tile([NT, 16], F32)
            nc.sync.dma_start(guard[:], hout_d[64:SHARD:128, 0:16])
            full = (NT - 1) * 128
            nc.sync.dma_start(
                hout_d[:full, 0:K_FAC].rearrange("(i p) k -> p i k", p=128),
                intall[:, : NT - 1, :],
            )
            nc.sync.dma_start(
                hout_d[full:, 0:K_FAC], intall[: SHARD - full, NT - 1, :]
            )

            # degV out: degv_all[p, i] -> degV[i*128 + p]
            nc.scalar.dma_start(
                degv_d[:full, :].rearrange("(i p) one -> p (i one)", p=128),
                degv_all[:, : NT - 1],
            )
            nc.scalar.dma_start(
                degv_d[full:, :], degv_all[: SHARD - full, NT - 1 : NT]
            )

            # ---------------- phase C: AllReduce partials, compute degE --------
            # acc_sb rows: cnt -> c, hi -> 9 + c, lo -> 18 + c
            acc_sb = cpool.tile([27, 512], F32)
            nc.vector.memset(acc_sb[0:27, :], 0.0)
            for c in range(9):
                w = 512 if c < 8 else K_FAC
                off = 32 * (c % 3)
                mir = spool.tile([128, 512], F32, tag="mir")
                nc.vector.tensor_copy(mir[off : off + 3, :w], acc_slice(c, w))
                for j in range(3):
                    nc.sync.dma_start(
                        acc_sb[9 * j + c : 9 * j + c + 1, :w],
                        mir[off + j : off + j + 1, :w],
                    )
            ar_in = dpool.tile([27, 512], F32)
            nc.gpsimd.dma_start(ar_in[:], acc_sb[0:27, :])
            ar_out = dpool.tile([27, 512], F32, addr_space="Shared")
            nc.gpsimd.collective_compute(
                "AllReduce",
                ALU.add,
                replica_groups=[list(range(NC))],
                ins=[ar_in[:].opt()],
                outs=[ar_out[:].opt()],
            )
            cnt_t = cpool.tile([9, 512], F32)
            nc.gpsimd.dma_start(cnt_t[0:9, :], ar_out[0:9, :])
            hi_t = cpool.tile([9, 512], F32)
            nc.gpsimd.dma_start(hi_t[0:9, :], ar_out[9:18, :])
            lo_t = cpool.tile([9, 512], F32)
            nc.gpsimd.dma_start(lo_t[0:9, :], ar_out[18:27, :])

            # wsum = 32*hi + lo; degE = sqrt(max(cnt,1) / wsum).  The int
            # chunk carries 2*cnt and 2*wsum; the factor cancels in the ratio.
            nc.vector.scalar_tensor_tensor(
                hi_t[0:9, :], hi_t[0:9, :], 32.0, lo_t[0:9, :],
                ALU.mult, ALU.add,
            )
            nc.vector.tensor_scalar_max(cnt_t[0:9, :], cnt_t[0:9, :], 1.0)
            # clamp: only affects the never-output padding cells of the int
            # row (wsum there is 0, and sqrt(inf) faults the scalar engine)
            nc.vector.tensor_scalar_max(hi_t[0:9, :], hi_t[0:9, :], 1e-30)
            nc.vector.reciprocal(hi_t[0:9, :], hi_t[0:9, :])
            rr = cpool.tile([9, 512], F32)
            nc.vector.tensor_tensor(rr[0:9, :], cnt_t[0:9, :], hi_t[0:9, :],
                                    ALU.mult)
            dege_all = cpool.tile([9, 512], F32)
            nc.scalar.sqrt(dege_all[0:9, :], rr[0:9, :])
            nc.scalar.dma_start(
                dege_d[K_FAC:, :].rearrange("(c k) one -> c (k one)", c=8),
                dege_all[0:8, :],
            )
            nc.scalar.dma_start(
                dege_d[0:K_FAC, :].rearrange("k one -> one k"),
                dege_all[8:9, 0:K_FAC],
            )

    nc.finalize()
    return nc


def _constants():
    ident = np.eye(128, dtype=np.float32)
    p = np.arange(128)
    gmat = ((p[:, None] // 2) % 8 == (p[None, :] // 2) % 8).astype(np.float32)
    return ident, gmat


def kernel(hidden, H, int_emb):
    global _CACHED
    if _CACHED is None:
        _CACHED = _build()
    nc = _CACHED
    hidden = np.ascontiguousarray(hidden, dtype=np.float32)
    H = np.ascontiguousarray(H, dtype=np.float32)
    int_emb = np.ascontiguousarray(int_emb, dtype=np.float32)
    ident, gmat = _constants()
    in_maps = [
        {
            "hidden": hidden[r * SHARD : (r + 1) * SHARD],
            "H": H[r * SHARD : (r + 1) * SHARD],
            "int_emb": int_emb,
            "ident": ident,
            "G": gmat,
        }
        for r in range(NC)
    ]
    res = bass_utils.run_bass_kernel_spmd(
        nc, in_maps, core_ids=list(range(NC)), trace=False
    )
    outs = res.results
    H_out = np.concatenate([outs[r]["Hout"] for r in range(NC)], axis=0)
    degV = np.concatenate([outs[r]["degV"] for r in range(NC)], axis=0)
    degE = outs[0]["degE"]
    return H_out, degV, degE


# revision 39
# speedup vs baseline: 1.0474x; 1.0474x over previous
"""Trainium2 Bass kernel for nn_DisentangleGraph (topk_masking).

Computes, for hidden (20000,256), H (20000,4096), int_emb (8,256):
  sim   = 10 * cosine(hidden, int_emb)                  (20000, 8)
  int_H = 2.0 where rank-within-column(sim) <= 6000     (top-6001 per column)
  H_out = concat([int_H, H], axis=1)                    (20000, 4104)
  degV  = rowsum(H_out);  degE = colmean of degV over nonzeros
  returns (H_out, degV**-0.5, degE**-0.5)

Sharding: node dimension split across 8 NeuronCores (2500 nodes each).
The per-column top-k threshold (the 6001-th largest sim value) is found with
a bisection over the value space after an AllGather of the (tiny) sim matrix;
degE needs an AllReduce of per-shard column sums.  Selection uses
`sim >= t` where t is the exact 6001-th largest value — equivalent to the
reference's double-argsort rank test whenever the threshold value is unique
in its column (holds for this input; verified against the reference).

Structure: the H stream (82 MB of DMA per core) must never wait for the
threshold search, so each H tile is copied to bf16 (exact for 0/1 data,
fused with the row-sum on the scalar engine) and written straight back out
to H_out[:, 8:].  The 8 int columns are staged and written once at the end;
a tiny read of H_out that overlaps both regions forces the int write after
every H write (odd rows share a 64-byte HBM line between the two, so they
must not overlap in time; Tile orders DMAs via DRAM range dependencies).
The column-sum matmuls run in bf16 (4x the fp32 PE rate) on the exact split
degV = 32*hi + lo; products and per-core partial sums stay below 2^24 so
nothing is rounded.
"""

import numpy as np

import concourse.bacc as bacc
import concourse.mybir as mybir
from concourse import bass_utils
from concourse.tile import TileContext

F32 = mybir.dt.float32
BF16 = mybir.dt.bfloat16
FP8 = mybir.dt.float8e4
ALU = mybir.AluOpType
ACTF = mybir.ActivationFunctionType
AX = mybir.AxisListType

N_NODES = 20000
NUM_EDGE = 4096
DIM = 256
K_FAC = 8
NC = 8
SHARD = N_NODES // NC          # 2500
NT = (SHARD + 127) // 128      # 20 tiles per shard
SEL_CNT = 6001.0               # rank <= floor(0.3*N) selects 6001 values
TEMP = 10.0
EPS = 1e-8
N_ITER = 31                    # bisection iterations (converges to exact fp32)
WIDTH = NUM_EDGE + K_FAC       # 4104
T_BUFS = 2
TB_BUFS = NT_B = 20

_CACHED = None


def _rows(i):
    return min(128, SHARD - i * 128)


def _build():
    nc = bacc.Bacc("TRN2", target_bir_lowering=False, debug=False, num_devices=NC)

    hid_d = nc.dram_tensor("hidden", [SHARD, DIM], F32, kind="ExternalInput")
    h_d = nc.dram_tensor("H", [SHARD, NUM_EDGE], F32, kind="ExternalInput")
    emb_d = nc.dram_tensor("int_emb", [K_FAC, DIM], F32, kind="ExternalInput")
    ident_d = nc.dram_tensor("ident", [128, 128], F32, kind="ExternalInput")
    g_d = nc.dram_tensor("G", [128, 128], F32, kind="ExternalInput")

    hout_d = nc.dram_tensor("Hout", [SHARD, WIDTH], F32, kind="ExternalOutput")
    degv_d = nc.dram_tensor("degV", [SHARD, 1], F32, kind="ExternalOutput")
    dege_d = nc.dram_tensor("degE", [WIDTH, 1], F32, kind="ExternalOutput")

    with TileContext(nc) as tc:
        with (
            tc.tile_pool(name="const", bufs=1) as cpool,
            tc.tile_pool(name="hid", bufs=3) as hpool,
            tc.tile_pool(name="hnt", bufs=3) as tpool_hnt,
            tc.tile_pool(name="bigT", bufs=T_BUFS) as Tpool,
            tc.tile_pool(name="bft", bufs=TB_BUFS) as Bpool,
            tc.tile_pool(name="small", bufs=3) as spool,
            tc.tile_pool(name="psA", bufs=1, space="PSUM") as psA,
            tc.tile_pool(name="psB", bufs=1, space="PSUM") as psB,
            tc.tile_pool(name="psC", bufs=1, space="PSUM") as psC,
            tc.tile_pool(name="dram", bufs=1, space="DRAM") as dpool,
        ):
            # ---------------- constants ----------------
            ident = cpool.tile([128, 128], F32)
            nc.scalar.dma_start(ident[:], ident_d[:])
            gmat = cpool.tile([128, 128], F32)
            nc.scalar.dma_start(gmat[:], g_d[:])
            ones_row = cpool.tile([1, 128], F32)
            nc.vector.memset(ones_row[:], 1.0)

            # ---------------- normalized int_emb, transposed ----------------
            emb = cpool.tile([K_FAC, DIM], F32)
            nc.scalar.dma_start(emb[:], emb_d[:])
            esq = cpool.tile([K_FAC, DIM], F32)
            ess = cpool.tile([K_FAC, 1], F32)
            nc.vector.tensor_tensor(esq[:], emb[:], emb[:], ALU.mult)
            nc.vector.reduce_sum(ess[:], esq[:], axis=AX.X)
            enorm = cpool.tile([K_FAC, 1], F32)
            nc.scalar.sqrt(enorm[:], ess[:])
            nc.vector.tensor_scalar_max(enorm[:], enorm[:], EPS)
            erin = cpool.tile([K_FAC, 1], F32)
            nc.vector.reciprocal(erin[:], enorm[:])
            en = cpool.tile([K_FAC, DIM], F32)
            nc.vector.tensor_scalar_mul(en[:], emb[:], erin[:])
            enT = cpool.tile([128, 2 * K_FAC], F32)  # chunk c at [:, c*8:(c+1)*8]
            for c in range(2):
                pt = psB.tile([128, 128], F32, tag="tr")
                nc.tensor.transpose(
                    pt[:128, :K_FAC], en[:, c * 128 : (c + 1) * 128],
                    ident[:K_FAC, :K_FAC],
                )
                nc.vector.tensor_copy(enT[:, c * K_FAC : (c + 1) * K_FAC],
                                      pt[:128, :K_FAC])

            # ---------------- phase A: local sim + transposed sim ----------------
            # (kept off the scalar engine: phase B's bf16 converts own it)
            sim_loc = cpool.tile([128, NT, K_FAC], F32)   # node-major local sim
            simT_loc = cpool.tile([K_FAC, SHARD], F32)    # column-major local sim
            for i in range(NT):
                rows = _rows(i)
                r0 = i * 128
                ht = hpool.tile([128, DIM], F32, tag="h")
                nc.scalar.dma_start(ht[:rows, :], hid_d[r0 : r0 + rows, :])
                sq = hpool.tile([128, DIM], F32, tag="sq")
                ss = spool.tile([128, 1], F32, tag="ss")
                nc.vector.tensor_tensor(sq[:rows, :], ht[:rows, :], ht[:rows, :],
                                        ALU.mult)
                nc.vector.reduce_sum(ss[:rows, :], sq[:rows, :], axis=AX.X)
                nrm = spool.tile([128, 1], F32, tag="nrm")
                nc.scalar.sqrt(nrm[:rows, :], ss[:rows, :])
                nc.vector.tensor_scalar_max(nrm[:rows, :], nrm[:rows, :], EPS)
                rin = spool.tile([128, 1], F32, tag="rin")
                nc.vector.reciprocal(rin[:rows, :], nrm[:rows, :])
                hn = hpool.tile([128, DIM], F32, tag="hn")
                nc.vector.tensor_scalar_mul(hn[:rows, :], ht[:rows, :], rin[:rows, :])

                hnT = tpool_hnt.tile([128, 256], F32, tag="hnT")
                for c in range(2):
                    pt = psB.tile([128, 128], F32, tag="tr")
                    nc.tensor.transpose(
                        pt[:128, :rows],
                        hn[:rows, c * 128 : (c + 1) * 128],
                        ident[:rows, :rows],
                    )
                    nc.vector.tensor_copy(hnT[:, c * 128 : c * 128 + rows],
                                          pt[:128, :rows])
                psim = psC.tile([128, 128], F32, tag="mm")
                for c in range(2):
                    nc.tensor.matmul(
                        psim[:rows, :K_FAC],
                        hnT[:, c * 128 : c * 128 + rows],
                        enT[:, c * K_FAC : (c + 1) * K_FAC],
                        start=(c == 0),
                        stop=(c == 1),
                    )
                # sim = TEMP * (hn @ en.T)
                nc.vector.tensor_scalar_mul(sim_loc[:rows, i, :],
                                            psim[:rows, :K_FAC], TEMP)
                ptT = psC.tile([128, 128], F32, tag="mm")
                nc.tensor.transpose(
                    ptT[:K_FAC, :rows], sim_loc[:rows, i, :], ident[:rows, :rows]
                )
                nc.vector.tensor_copy(simT_loc[:, r0 : r0 + rows],
                                      ptT[:K_FAC, :rows])

            # ---------------- AllGather sim ----------------
            simT_d = dpool.tile([K_FAC, SHARD], F32)
            nc.gpsimd.dma_start(simT_d[:], simT_loc[:])
            simfull_d = dpool.tile([NC * K_FAC, SHARD], F32, addr_space="Shared")
            nc.gpsimd.collective_compute(
                "AllGather",
                ALU.bypass,
                replica_groups=[list(range(NC))],
                ins=[simT_d[:].opt()],
                outs=[simfull_d[:].opt()],
            )
            # grouped layout: partition p = (q, h), q = rank*8+f, h in {0,1}
            # -> column of partition p is (p//2) % 8; 16 partitions per column.
            sim_g = cpool.tile([128, SHARD // 2], F32)
            nc.scalar.dma_start(
                sim_g[:], simfull_d[:].rearrange("q (h i) -> (q h) i", h=2)
            )

            # ---------------- bisection for per-column threshold ----------------
            # state: interval [lo, lo + 2*hw); probe mid = lo + hw; on
            # count >= target: lo = mid; always hw /= 2.  lo converges to the
            # exact fp32 value of the 6001-th largest element per column.
            lo = cpool.tile([128, 1], F32)
            hw = cpool.tile([128, 1], F32)
            mid = cpool.tile([128, 1], F32)
            nc.vector.memset(lo[:], -10.5)
            nc.vector.memset(hw[:], 10.5)
            nc.vector.memset(mid[:], 0.0)
            cmp_buf = cpool.tile([128, SHARD // 2], BF16)
            for it in range(N_ITER):
                pcnt = spool.tile([128, 1], F32, tag="pcnt")
                nc.vector.tensor_scalar(
                    out=cmp_buf[:],
                    in0=sim_g[:],
                    scalar1=mid[:],
                    scalar2=None,
                    op0=ALU.is_ge,
                    op1=ALU.add,
                    accum_out=pcnt[:],
                )
                pc = psC.tile([128, 128], F32, tag="mm")
                nc.tensor.matmul(pc[:128, :1], gmat[:], pcnt[:], start=True,
                                 stop=True)
                cnt = spool.tile([128, 1], F32, tag="cnt")
                nc.vector.tensor_copy(cnt[:], pc[:128, 0:1])
                geqf = spool.tile([128, 1], F32, tag="geqf")
                nc.vector.tensor_scalar(
                    out=geqf[:], in0=cnt[:], scalar1=SEL_CNT, scalar2=None,
                    op0=ALU.is_ge,
                )
                d = spool.tile([128, 1], F32, tag="d")
                nc.vector.tensor_tensor(d[:], geqf[:], hw[:], ALU.mult)
                nc.vector.tensor_tensor(lo[:], lo[:], d[:], ALU.add)
                nc.vector.tensor_scalar_mul(hw[:], hw[:], 0.5)
                nc.vector.tensor_tensor(mid[:], lo[:], hw[:], ALU.add)

            # threshold row (1, 8): column f lives (a.o.) on partition 2f
            th_row = cpool.tile([1, K_FAC], F32)
            nc.scalar.dma_start(th_row[:], lo[0:16:2, :])
            pbc = psC.tile([128, 128], F32, tag="mm")
            nc.tensor.matmul(pbc[:128, :K_FAC], ones_row[:], th_row[:],
                             start=True, stop=True)
            thr = cpool.tile([128, K_FAC], F32)
            nc.vector.tensor_copy(thr[:], pbc[:128, :K_FAC])

            # ---------------- phase B: stream H, build H_out, accumulate sums ----
            # Column-sum accumulators: chunk c in {0..8} (8 H chunks of 512
            # cols + the 8 int cols) accumulates rows [colsum(mask),
            # colsum(mask*hi), colsum(mask*lo)] as a (3, 512) PSUM region,
            # where degV = 32*hi + lo.  PE matmul outputs must start at
            # partition 0/32/64: chunk c -> bank paccs[c // 3], offset
            # 32 * (c % 3).
            paccs = [
                psA.tile([128, 512], F32, tag=f"pacc{t}", name=f"pacc{t}")
                for t in range(6)
            ]

            def acc_slice(c, width=512):
                return paccs[c // 3][32 * (c % 3) : 32 * (c % 3) + 3, :width]

            def acc_slice2(c, width=512):
                return paccs[3 + c // 3][32 * (c % 3) : 32 * (c % 3) + 3, :width]

            degv_all = cpool.tile([128, NT], F32)
            intall = cpool.tile([128, NT, K_FAC], F32)
            for i in range(NT):
                rows = _rows(i)
                r0 = i * 128
                T = Tpool.tile([128, NUM_EDGE], F32, tag="T")
                nc.sync.dma_start(T[:rows, :], h_d[r0 : r0 + rows, :])
                # H part of H_out goes out immediately — it does not depend
                # on the threshold search.
                nc.scalar.dma_start(hout_d[r0 : r0 + rows, K_FAC:], T[:rows, :])
                # f32 -> bf16 copy of H (exact: values are 0/1), fused with
                # the row-sum accumulation, on the scalar engine
                Tb = Bpool.tile([128, NUM_EDGE], FP8, tag="Tb")
                par = spool.tile([128, 8], F32, tag="par")
                for c in range(8):
                    nc.scalar.activation(
                        Tb[:rows, 512 * c : 512 * (c + 1)],
                        T[:rows, 512 * c : 512 * (c + 1)],
                        ACTF.Copy,
                        accum_out=par[:rows, c : c + 1],
                    )
                r = spool.tile([128, 1], F32, tag="r")
                nc.vector.reduce_sum(r[:rows, :], par[:rows, :], axis=AX.X)
                sel = spool.tile([128, K_FAC], F32, tag="sel")
                nc.vector.tensor_tensor(
                    sel[:rows, :], sim_loc[:rows, i, :], thr[:rows, :], ALU.is_ge
                )
                s2 = spool.tile([128, 1], F32, tag="s2")
                nc.scalar.activation(
                    intall[:rows, i, :], sel[:rows, :], ACTF.Copy, scale=2.0,
                    accum_out=s2[:rows, :],
                )
                Tbi = spool.tile([128, K_FAC], FP8, tag="Tbi")
                nc.scalar.activation(Tbi[:rows, :], sel[:rows, :], ACTF.Copy,
                                     scale=2.0)
                dv = spool.tile([128, 1], F32, tag="dv")
                nc.vector.tensor_tensor(dv[:rows, :], r[:rows, :], s2[:rows, :],
                                        ALU.add)
                # base-16 digits of degV (dv <= 4112, exact integer):
                # dv = 4096*dA + 256*dB + 16*dC + dD, all digits <= 16 so
                # every one is fp8e4-exact.  floor(x/s) computed as
                # round-to-nearest(x/s - 0.499) (x/s sits on a 1/s grid).
                dA = spool.tile([128, 1], F32, tag="dA")
                nc.vector.tensor_scalar(
                    out=dA[:rows, :], in0=dv[:rows, :], scalar1=4096.0,
                    scalar2=None, op0=ALU.is_ge,
                )
                remA = spool.tile([128, 1], F32, tag="remA")
                nc.vector.scalar_tensor_tensor(
                    remA[:rows, :], dA[:rows, :], -4096.0, dv[:rows, :],
                    ALU.mult, ALU.add,
                )
                digs = [dA]
                rem = remA
                for shift, tg in ((512, "B"), (64, "C"), (8, "D")):
                    t2 = spool.tile([128, 1], F32, tag=f"t2{tg}", name=f"t2{tg}")
                    nc.vector.tensor_scalar(
                        out=t2[:rows, :], in0=rem[:rows, :],
                        scalar1=1.0 / shift, scalar2=-0.499,
                        op0=ALU.mult, op1=ALU.add,
                    )
                    d_i = spool.tile([128, 1], mybir.dt.int32, tag=f"di{tg}",
                                     name=f"di{tg}")
                    nc.vector.tensor_copy(d_i[:rows, :], t2[:rows, :])
                    d_f = spool.tile([128, 1], F32, tag=f"df{tg}",
                                     name=f"df{tg}")
                    nc.vector.tensor_copy(d_f[:rows, :], d_i[:rows, :])
                    rem2 = spool.tile([128, 1], F32, tag=f"rm{tg}",
                                      name=f"rm{tg}")
                    nc.vector.scalar_tensor_tensor(
                        rem2[:rows, :], d_f[:rows, :], -float(shift),
                        rem[:rows, :], ALU.mult, ALU.add,
                    )
                    digs.append(d_f)
                    rem = rem2
                lw = spool.tile([128, 3], FP8, tag="lw")
                nc.vector.memset(lw[:rows, 0:1], 1.0)
                nc.scalar.activation(lw[:rows, 1:2], digs[0][:rows, :], ACTF.Copy)
                nc.scalar.activation(lw[:rows, 2:3], digs[1][:rows, :], ACTF.Copy)
                lw2 = spool.tile([128, 3], FP8, tag="lw2")
                nc.scalar.activation(lw2[:rows, 0:1], digs[2][:rows, :], ACTF.Copy)
                nc.scalar.activation(lw2[:rows, 1:2], digs[3][:rows, :], ACTF.Copy)
                nc.scalar.activation(lw2[:rows, 2:3], rem[:rows, :], ACTF.Copy)
                for c in range(8):
                    nc.tensor.matmul(
                        acc_slice(c),
                        lw[:rows, :],
                        Tb[:rows, 512 * c : 512 * (c + 1)],
                        start=(i == 0),
                        stop=(i == NT - 1),
                    )
                    nc.tensor.matmul(
                        acc_slice2(c),
                        lw2[:rows, :],
                        Tb[:rows, 512 * c : 512 * (c + 1)],
                        start=(i == 0),
                        stop=(i == NT - 1),
                    )
                nc.tensor.matmul(
                    acc_slice(8, K_FAC),
                    lw[:rows, :],
                    Tbi[:rows, :],
                    start=(i == 0),
                    stop=(i == NT - 1),
                )
                nc.tensor.matmul(
                    acc_slice2(8, K_FAC),
                    lw2[:rows, :],
                    Tbi[:rows, :],
                    start=(i == 0),
                    stop=(i == NT - 1),
                )
                rec = spool.tile([128, 1], F32, tag="rec")
                nc.vector.reciprocal(rec[:rows, :], dv[:rows, :])
                nc.scalar.sqrt(degv_all[:rows, i : i + 1], rec[:rows, :])

            # ---- int columns of H_out, written once, after every H write ----
            # Odd rows share a 64-byte HBM line with the H-part write of the
            # previous row, so the int write must not overlap any H write in
            # time.  The guard read overlaps every tile's H region (RAW: it
            # waits for all H writes) and the int region (WAR: the int write
            # waits for it).
            guard = cpool.tile([NT, 16], F32)
            nc.sync.dma_start(guard[:], hout_d[64:SHARD:128, 0:16])
            full = (NT - 1) * 128
            nc.sync.dma_start(
                hout_d[:full, 0:K_FAC].rearrange("(i p) k -> p i k", p=128),
                intall[:, : NT - 1, :],
            )
            nc.sync.dma_start(
                hout_d[full:, 0:K_FAC], intall[: SHARD - full, NT - 1, :]
            )

            # degV out: degv_all[p, i] -> degV[i*128 + p]
            nc.scalar.dma_start(
                degv_d[:full, :].rearrange("(i p) one -> p (i one)", p=128),
                degv_all[:, : NT - 1],
            )
            nc.scalar.dma_start(
                degv_d[full:, :], degv_all[: SHARD - full, NT - 1 : NT]
            )

            # ---------------- phase C: AllReduce partials, compute degE --------
            # acc_sb rows: cnt -> c, dA -> 9+c, dB -> 18+c, dC -> 27+c,
            # dD -> 36+c
            acc_sb = cpool.tile([54, 512], F32)
            nc.vector.memset(acc_sb[0:54, :], 0.0)
            for c in range(9):
                w = 512 if c < 8 else K_FAC
                off = 32 * (c % 3)
                mir = spool.tile([128, 512], F32, tag="mir")
                nc.vector.tensor_copy(mir[off : off + 3, :w], acc_slice(c, w))
                mir2 = spool.tile([128, 512], F32, tag="mir2")
                nc.vector.tensor_copy(mir2[off : off + 3, :w], acc_slice2(c, w))
                for j in range(3):
                    nc.sync.dma_start(
                        acc_sb[9 * j + c : 9 * j + c + 1, :w],
                        mir[off + j : off + j + 1, :w],
                    )
                for j in range(3):
                    nc.sync.dma_start(
                        acc_sb[27 + 9 * j + c : 27 + 9 * j + c + 1, :w],
                        mir2[off + j : off + j + 1, :w],
                    )
            ar_in = dpool.tile([54, 512], F32)
            nc.gpsimd.dma_start(ar_in[:], acc_sb[0:54, :])
            ar_out = dpool.tile([54, 512], F32, addr_space="Shared")
            nc.gpsimd.collective_compute(
                "AllReduce",
                ALU.add,
                replica_groups=[list(range(NC))],
                ins=[ar_in[:].opt()],
                outs=[ar_out[:].opt()],
            )
            cnt_t = cpool.tile([9, 512], F32)
            nc.gpsimd.dma_start(cnt_t[0:9, :], ar_out[0:9, :])
            hi_t = cpool.tile([9, 512], F32)
            nc.gpsimd.dma_start(hi_t[0:9, :], ar_out[9:18, :])
            lo_t = cpool.tile([9, 512], F32)
            nc.gpsimd.dma_start(lo_t[0:9, :], ar_out[18:27, :])
            sc_t = cpool.tile([9, 512], F32)
            nc.gpsimd.dma_start(sc_t[0:9, :], ar_out[27:36, :])
            sd_t = cpool.tile([9, 512], F32)
            nc.gpsimd.dma_start(sd_t[0:9, :], ar_out[36:45, :])
            se_t = cpool.tile([9, 512], F32)
            nc.gpsimd.dma_start(se_t[0:9, :], ar_out[45:54, :])

            # wsum = 4096*dA + 256*dB + 16*dC + dD via Horner; degE =
            # sqrt(max(cnt,1) / wsum).  The int chunk carries 2x everything;
            # the factor cancels in the ratio.
            nc.vector.scalar_tensor_tensor(
                hi_t[0:9, :], hi_t[0:9, :], 8.0, lo_t[0:9, :],
                ALU.mult, ALU.add,
            )
            nc.vector.scalar_tensor_tensor(
                hi_t[0:9, :], hi_t[0:9, :], 8.0, sc_t[0:9, :],
                ALU.mult, ALU.add,
            )
            nc.vector.scalar_tensor_tensor(
                hi_t[0:9, :], hi_t[0:9, :], 8.0, sd_t[0:9, :],
                ALU.mult, ALU.add,
            )
            nc.vector.scalar_tensor_tensor(
                hi_t[0:9, :], hi_t[0:9, :], 8.0, se_t[0:9, :],
                ALU.mult, ALU.add,
            )
            nc.vector.tensor_scalar_max(cnt_t[0:9, :], cnt_t[0:9, :], 1.0)
            # clamp: only affects the never-output padding cells of the int
            # row (wsum there is 0, and sqrt(inf) faults the scalar engine)
            nc.vector.tensor_scalar_max(hi_t[0:9, :], hi_t[0:9, :], 1e-30)
            nc.vector.reciprocal(hi_t[0:9, :], hi_t[0:9, :])
            rr = cpool.tile([9, 512], F32)
            nc.vector.tensor_tensor(rr[0:9, :], cnt_t[0:9, :], hi_t[0:9, :],
                                    ALU.mult)
            dege_all = cpool.tile([9, 512], F32)
            nc.scalar.sqrt(dege_all[0:9, :], rr[0:9, :])
            nc.scalar.dma_start(
                dege_d[K_FAC:, :].rearrange("(c k) one -> c (k one)", c=8),
                dege_all[0:8, :],
            )
            nc.scalar.dma_start(
                dege_d[0:K_FAC, :].rearrange("k one -> one k"),
                dege_all[8:9, 0:K_FAC],
            )

    nc.finalize()
    return nc


def _constants():
    ident = np.eye(128, dtype=np.float32)
    p = np.arange(128)
    gmat = ((p[:, None] // 2) % 8 == (p[None, :] // 2) % 8).astype(np.float32)
    return ident, gmat


def kernel(hidden, H, int_emb):
    global _CACHED
    if _CACHED is None:
        _CACHED = _build()
    nc = _CACHED
    hidden = np.ascontiguousarray(hidden, dtype=np.float32)
    H = np.ascontiguousarray(H, dtype=np.float32)
    int_emb = np.ascontiguousarray(int_emb, dtype=np.float32)
    ident, gmat = _constants()
    in_maps = [
        {
            "hidden": hidden[r * SHARD : (r + 1) * SHARD],
            "H": H[r * SHARD : (r + 1) * SHARD],
            "int_emb": int_emb,
            "ident": ident,
            "G": gmat,
        }
        for r in range(NC)
    ]
    res = bass_utils.run_bass_kernel_spmd(
        nc, in_maps, core_ids=list(range(NC)), trace=False
    )
    outs = res.results
    H_out = np.concatenate([outs[r]["Hout"] for r in range(NC)], axis=0)
    degV = np.concatenate([outs[r]["degV"] for r in range(NC)], axis=0)
    degE = outs[0]["degE"]
    return H_out, degV, degE


# revision 40
# speedup vs baseline: 1.0739x; 1.0253x over previous
"""Trainium2 Bass kernel for nn_DisentangleGraph (topk_masking).

Computes, for hidden (20000,256), H (20000,4096), int_emb (8,256):
  sim   = 10 * cosine(hidden, int_emb)                  (20000, 8)
  int_H = 2.0 where rank-within-column(sim) <= 6000     (top-6001 per column)
  H_out = concat([int_H, H], axis=1)                    (20000, 4104)
  degV  = rowsum(H_out);  degE = colmean of degV over nonzeros
  returns (H_out, degV**-0.5, degE**-0.5)

Sharding: node dimension split across 8 NeuronCores (2500 nodes each).
The per-column top-k threshold (the 6001-th largest sim value) is found with
a bisection over the value space after an AllGather of the (tiny) sim matrix;
degE needs an AllReduce of per-shard column sums.  Selection uses
`sim >= t` where t is the exact 6001-th largest value — equivalent to the
reference's double-argsort rank test whenever the threshold value is unique
in its column (holds for this input; verified against the reference).

Structure: the H stream (82 MB of DMA per core) must never wait for the
threshold search, so each H tile is copied to bf16 (exact for 0/1 data,
fused with the row-sum on the scalar engine) and written straight back out
to H_out[:, 8:].  The 8 int columns are staged and written once at the end;
a tiny read of H_out that overlaps both regions forces the int write after
every H write (odd rows share a 64-byte HBM line between the two, so they
must not overlap in time; Tile orders DMAs via DRAM range dependencies).
The column-sum matmuls run in bf16 (4x the fp32 PE rate) on the exact split
degV = 32*hi + lo; products and per-core partial sums stay below 2^24 so
nothing is rounded.
"""

import numpy as np

import concourse.bacc as bacc
import concourse.mybir as mybir
from concourse import bass_utils
from concourse.tile import TileContext

F32 = mybir.dt.float32
BF16 = mybir.dt.bfloat16
FP8 = mybir.dt.float8e4
ALU = mybir.AluOpType
ACTF = mybir.ActivationFunctionType
AX = mybir.AxisListType

N_NODES = 20000
NUM_EDGE = 4096
DIM = 256
K_FAC = 8
NC = 8
SHARD = N_NODES // NC          # 2500
NT = (SHARD + 127) // 128      # 20 tiles per shard
SEL_CNT = 6001.0               # rank <= floor(0.3*N) selects 6001 values
TEMP = 10.0
EPS = 1e-8
N_ITER = 31                    # bisection iterations (converges to exact fp32)
WIDTH = NUM_EDGE + K_FAC       # 4104
T_BUFS = 2
TB_BUFS = NT_B = 20

_CACHED = None


def _rows(i):
    return min(128, SHARD - i * 128)


def _build():
    nc = bacc.Bacc("TRN2", target_bir_lowering=False, debug=False, num_devices=NC)

    hid_d = nc.dram_tensor("hidden", [SHARD, DIM], F32, kind="ExternalInput")
    h_d = nc.dram_tensor("H", [SHARD, NUM_EDGE], F32, kind="ExternalInput")
    emb_d = nc.dram_tensor("int_emb", [K_FAC, DIM], F32, kind="ExternalInput")
    ident_d = nc.dram_tensor("ident", [128, 128], F32, kind="ExternalInput")
    g_d = nc.dram_tensor("G", [128, 128], F32, kind="ExternalInput")

    hout_d = nc.dram_tensor("Hout", [SHARD, WIDTH], F32, kind="ExternalOutput")
    degv_d = nc.dram_tensor("degV", [SHARD, 1], F32, kind="ExternalOutput")
    dege_d = nc.dram_tensor("degE", [WIDTH, 1], F32, kind="ExternalOutput")

    with TileContext(nc) as tc:
        with (
            tc.tile_pool(name="const", bufs=1) as cpool,
            tc.tile_pool(name="hid", bufs=3) as hpool,
            tc.tile_pool(name="hnt", bufs=3) as tpool_hnt,
            tc.tile_pool(name="bigT", bufs=T_BUFS) as Tpool,
            tc.tile_pool(name="bft", bufs=TB_BUFS) as Bpool,
            tc.tile_pool(name="small", bufs=3) as spool,
            tc.tile_pool(name="psA", bufs=1, space="PSUM") as psA,
            tc.tile_pool(name="psB", bufs=1, space="PSUM") as psB,
            tc.tile_pool(name="psC", bufs=1, space="PSUM") as psC,
            tc.tile_pool(name="dram", bufs=1, space="DRAM") as dpool,
        ):
            # ---------------- constants ----------------
            ident = cpool.tile([128, 128], F32)
            nc.scalar.dma_start(ident[:], ident_d[:])
            gmat = cpool.tile([128, 128], F32)
            nc.scalar.dma_start(gmat[:], g_d[:])
            ones_row = cpool.tile([1, 128], F32)
            nc.vector.memset(ones_row[:], 1.0)

            # ---------------- normalized int_emb, transposed ----------------
            emb = cpool.tile([K_FAC, DIM], F32)
            nc.scalar.dma_start(emb[:], emb_d[:])
            esq = cpool.tile([K_FAC, DIM], F32)
            ess = cpool.tile([K_FAC, 1], F32)
            nc.vector.tensor_tensor(esq[:], emb[:], emb[:], ALU.mult)
            nc.vector.reduce_sum(ess[:], esq[:], axis=AX.X)
            enorm = cpool.tile([K_FAC, 1], F32)
            nc.scalar.sqrt(enorm[:], ess[:])
            nc.vector.tensor_scalar_max(enorm[:], enorm[:], EPS)
            erin = cpool.tile([K_FAC, 1], F32)
            nc.vector.reciprocal(erin[:], enorm[:])
            en = cpool.tile([K_FAC, DIM], F32)
            nc.vector.tensor_scalar_mul(en[:], emb[:], erin[:])
            enT = cpool.tile([128, 2 * K_FAC], F32)  # chunk c at [:, c*8:(c+1)*8]
            for c in range(2):
                pt = psB.tile([128, 128], F32, tag="tr")
                nc.tensor.transpose(
                    pt[:128, :K_FAC], en[:, c * 128 : (c + 1) * 128],
                    ident[:K_FAC, :K_FAC],
                )
                nc.vector.tensor_copy(enT[:, c * K_FAC : (c + 1) * K_FAC],
                                      pt[:128, :K_FAC])

            # ---------------- phase A: local sim + transposed sim ----------------
            # (kept off the scalar engine: phase B's bf16 converts own it)
            sim_loc = cpool.tile([128, NT, K_FAC], F32)   # node-major local sim
            simT_loc = cpool.tile([K_FAC, SHARD], F32)    # column-major local sim
            for i in range(NT):
                rows = _rows(i)
                r0 = i * 128
                ht = hpool.tile([128, DIM], F32, tag="h")
                nc.scalar.dma_start(ht[:rows, :], hid_d[r0 : r0 + rows, :])
                sq = hpool.tile([128, DIM], F32, tag="sq")
                ss = spool.tile([128, 1], F32, tag="ss")
                nc.vector.tensor_tensor(sq[:rows, :], ht[:rows, :], ht[:rows, :],
                                        ALU.mult)
                nc.vector.reduce_sum(ss[:rows, :], sq[:rows, :], axis=AX.X)
                nrm = spool.tile([128, 1], F32, tag="nrm")
                nc.scalar.sqrt(nrm[:rows, :], ss[:rows, :])
                nc.vector.tensor_scalar_max(nrm[:rows, :], nrm[:rows, :], EPS)
                rin = spool.tile([128, 1], F32, tag="rin")
                nc.vector.reciprocal(rin[:rows, :], nrm[:rows, :])
                hn = hpool.tile([128, DIM], F32, tag="hn")
                nc.vector.tensor_scalar_mul(hn[:rows, :], ht[:rows, :], rin[:rows, :])

                hnT = tpool_hnt.tile([128, 256], F32, tag="hnT")
                for c in range(2):
                    pt = psB.tile([128, 128], F32, tag="tr")
                    nc.tensor.transpose(
                        pt[:128, :rows],
                        hn[:rows, c * 128 : (c + 1) * 128],
                        ident[:rows, :rows],
                    )
                    nc.vector.tensor_copy(hnT[:, c * 128 : c * 128 + rows],
                                          pt[:128, :rows])
                psim = psC.tile([128, 128], F32, tag="mm")
                for c in range(2):
                    nc.tensor.matmul(
                        psim[:rows, :K_FAC],
                        hnT[:, c * 128 : c * 128 + rows],
                        enT[:, c * K_FAC : (c + 1) * K_FAC],
                        start=(c == 0),
                        stop=(c == 1),
                    )
                # sim = TEMP * (hn @ en.T)
                nc.vector.tensor_scalar_mul(sim_loc[:rows, i, :],
                                            psim[:rows, :K_FAC], TEMP)
                ptT = psC.tile([128, 128], F32, tag="mm")
                nc.tensor.transpose(
                    ptT[:K_FAC, :rows], sim_loc[:rows, i, :], ident[:rows, :rows]
                )
                nc.vector.tensor_copy(simT_loc[:, r0 : r0 + rows],
                                      ptT[:K_FAC, :rows])

            # ---------------- AllGather sim ----------------
            simT_d = dpool.tile([K_FAC, SHARD], F32)
            nc.gpsimd.dma_start(simT_d[:], simT_loc[:])
            simfull_d = dpool.tile([NC * K_FAC, SHARD], F32, addr_space="Shared")
            nc.gpsimd.collective_compute(
                "AllGather",
                ALU.bypass,
                replica_groups=[list(range(NC))],
                ins=[simT_d[:].opt()],
                outs=[simfull_d[:].opt()],
            )
            # grouped layout: partition p = (q, h), q = rank*8+f, h in {0,1}
            # -> column of partition p is (p//2) % 8; 16 partitions per column.
            sim_g = cpool.tile([128, SHARD // 2], F32)
            nc.scalar.dma_start(
                sim_g[:], simfull_d[:].rearrange("q (h i) -> (q h) i", h=2)
            )

            # ---------------- bisection for per-column threshold ----------------
            # state: interval [lo, lo + 2*hw); probe mid = lo + hw; on
            # count >= target: lo = mid; always hw /= 2.  lo converges to the
            # exact fp32 value of the 6001-th largest element per column.
            lo = cpool.tile([128, 1], F32)
            hw = cpool.tile([128, 1], F32)
            mid = cpool.tile([128, 1], F32)
            nc.vector.memset(lo[:], -10.5)
            nc.vector.memset(hw[:], 10.5)
            nc.vector.memset(mid[:], 0.0)
            cmp_buf = cpool.tile([128, SHARD // 2], BF16)
            for it in range(N_ITER):
                pcnt = spool.tile([128, 1], F32, tag="pcnt")
                nc.vector.tensor_scalar(
                    out=cmp_buf[:],
                    in0=sim_g[:],
                    scalar1=mid[:],
                    scalar2=None,
                    op0=ALU.is_ge,
                    op1=ALU.add,
                    accum_out=pcnt[:],
                )
                pc = psC.tile([128, 128], F32, tag="mm")
                nc.tensor.matmul(pc[:128, :1], gmat[:], pcnt[:], start=True,
                                 stop=True)
                cnt = spool.tile([128, 1], F32, tag="cnt")
                nc.vector.tensor_copy(cnt[:], pc[:128, 0:1])
                geqf = spool.tile([128, 1], F32, tag="geqf")
                nc.vector.tensor_scalar(
                    out=geqf[:], in0=cnt[:], scalar1=SEL_CNT, scalar2=None,
                    op0=ALU.is_ge,
                )
                d = spool.tile([128, 1], F32, tag="d")
                nc.vector.tensor_tensor(d[:], geqf[:], hw[:], ALU.mult)
                nc.vector.tensor_tensor(lo[:], lo[:], d[:], ALU.add)
                nc.vector.tensor_scalar_mul(hw[:], hw[:], 0.5)
                nc.vector.tensor_tensor(mid[:], lo[:], hw[:], ALU.add)

            # threshold row (1, 8): column f lives (a.o.) on partition 2f
            th_row = cpool.tile([1, K_FAC], F32)
            nc.scalar.dma_start(th_row[:], lo[0:16:2, :])
            pbc = psC.tile([128, 128], F32, tag="mm")
            nc.tensor.matmul(pbc[:128, :K_FAC], ones_row[:], th_row[:],
                             start=True, stop=True)
            thr = cpool.tile([128, K_FAC], F32)
            nc.vector.tensor_copy(thr[:], pbc[:128, :K_FAC])

            # ---------------- phase B: stream H, build H_out, accumulate sums ----
            # Column-sum accumulators: chunk c in {0..8} (8 H chunks of 512
            # cols + the 8 int cols) accumulates rows [colsum(mask),
            # colsum(mask*hi), colsum(mask*lo)] as a (3, 512) PSUM region,
            # where degV = 32*hi + lo.  PE matmul outputs must start at
            # partition 0/32/64: chunk c -> bank paccs[c // 3], offset
            # 32 * (c % 3).
            paccs = [
                psA.tile([128, 512], F32, tag=f"pacc{t}", name=f"pacc{t}")
                for t in range(6)
            ]

            def acc_slice(c, width=512):
                return paccs[c // 3][32 * (c % 3) : 32 * (c % 3) + 3, :width]

            def acc_slice2(c, width=512):
                return paccs[3 + c // 3][32 * (c % 3) : 32 * (c % 3) + 3, :width]

            degv_all = cpool.tile([128, NT], F32)
            intall = cpool.tile([128, NT, K_FAC], F32)
            r_all = cpool.tile([128, NT], F32)
            Tbs = []
            # loop 1: everything threshold-independent, emitted first so no
            # threshold-gated op ever sits ahead of a convert in the scalar
            # engine's FIFO queue (head-of-line blocking froze the stream).
            for i in range(NT):
                rows = _rows(i)
                r0 = i * 128
                T = Tpool.tile([128, NUM_EDGE], F32, tag="T")
                nc.sync.dma_start(T[:rows, :], h_d[r0 : r0 + rows, :])
                nc.scalar.dma_start(hout_d[r0 : r0 + rows, K_FAC:], T[:rows, :])
                Tb = Bpool.tile([128, NUM_EDGE], FP8, tag="Tb", name=f"Tb{i}")
                par = spool.tile([128, 8], F32, tag="par")
                for c in range(8):
                    nc.scalar.activation(
                        Tb[:rows, 512 * c : 512 * (c + 1)],
                        T[:rows, 512 * c : 512 * (c + 1)],
                        ACTF.Copy,
                        accum_out=par[:rows, c : c + 1],
                    )
                nc.vector.reduce_sum(r_all[:rows, i : i + 1], par[:rows, :],
                                     axis=AX.X)
                Tbs.append(Tb)
            # loop 2: threshold-dependent work, drains the resident fp8 tiles
            for i in range(NT):
                rows = _rows(i)
                Tb = Tbs[i]
                sel = spool.tile([128, K_FAC], F32, tag="sel")
                nc.vector.tensor_tensor(
                    sel[:rows, :], sim_loc[:rows, i, :], thr[:rows, :], ALU.is_ge
                )
                s2 = spool.tile([128, 1], F32, tag="s2")
                nc.scalar.activation(
                    intall[:rows, i, :], sel[:rows, :], ACTF.Copy, scale=2.0,
                    accum_out=s2[:rows, :],
                )
                Tbi = spool.tile([128, K_FAC], FP8, tag="Tbi")
                nc.scalar.activation(Tbi[:rows, :], sel[:rows, :], ACTF.Copy,
                                     scale=2.0)
                dv = spool.tile([128, 1], F32, tag="dv")
                nc.vector.tensor_tensor(dv[:rows, :], r_all[:rows, i : i + 1],
                                        s2[:rows, :], ALU.add)
                # base-8 digits of degV (dv <= 4112, exact integer): all
                # digits <= 7, fp8-exact.  floor(x/s) via rne(x/s - 0.499).
                dA = spool.tile([128, 1], F32, tag="dA")
                nc.vector.tensor_scalar(
                    out=dA[:rows, :], in0=dv[:rows, :], scalar1=4096.0,
                    scalar2=None, op0=ALU.is_ge,
                )
                remA = spool.tile([128, 1], F32, tag="remA")
                nc.vector.scalar_tensor_tensor(
                    remA[:rows, :], dA[:rows, :], -4096.0, dv[:rows, :],
                    ALU.mult, ALU.add,
                )
                digs = [dA]
                rem = remA
                for shift, tg in ((512, "B"), (64, "C"), (8, "D")):
                    t2 = spool.tile([128, 1], F32, tag=f"t2{tg}", name=f"t2{tg}")
                    nc.vector.tensor_scalar(
                        out=t2[:rows, :], in0=rem[:rows, :],
                        scalar1=1.0 / shift, scalar2=-0.499,
                        op0=ALU.mult, op1=ALU.add,
                    )
                    d_i = spool.tile([128, 1], mybir.dt.int32, tag=f"di{tg}",
                                     name=f"di{tg}")
                    nc.vector.tensor_copy(d_i[:rows, :], t2[:rows, :])
                    d_f = spool.tile([128, 1], F32, tag=f"df{tg}",
                                     name=f"df{tg}")
                    nc.vector.tensor_copy(d_f[:rows, :], d_i[:rows, :])
                    rem2 = spool.tile([128, 1], F32, tag=f"rm{tg}",
                                      name=f"rm{tg}")
                    nc.vector.scalar_tensor_tensor(
                        rem2[:rows, :], d_f[:rows, :], -float(shift),
                        rem[:rows, :], ALU.mult, ALU.add,
                    )
                    digs.append(d_f)
                    rem = rem2
                lw = spool.tile([128, 3], FP8, tag="lw")
                nc.vector.memset(lw[:rows, 0:1], 1.0)
                nc.scalar.activation(lw[:rows, 1:2], digs[0][:rows, :], ACTF.Copy)
                nc.scalar.activation(lw[:rows, 2:3], digs[1][:rows, :], ACTF.Copy)
                lw2 = spool.tile([128, 3], FP8, tag="lw2")
                nc.scalar.activation(lw2[:rows, 0:1], digs[2][:rows, :], ACTF.Copy)
                nc.scalar.activation(lw2[:rows, 1:2], digs[3][:rows, :], ACTF.Copy)
                nc.scalar.activation(lw2[:rows, 2:3], rem[:rows, :], ACTF.Copy)
                for c in range(8):
                    nc.tensor.matmul(
                        acc_slice(c),
                        lw[:rows, :],
                        Tb[:rows, 512 * c : 512 * (c + 1)],
                        start=(i == 0),
                        stop=(i == NT - 1),
                    )
                    nc.tensor.matmul(
                        acc_slice2(c),
                        lw2[:rows, :],
                        Tb[:rows, 512 * c : 512 * (c + 1)],
                        start=(i == 0),
                        stop=(i == NT - 1),
                    )
                nc.tensor.matmul(
                    acc_slice(8, K_FAC),
                    lw[:rows, :],
                    Tbi[:rows, :],
                    start=(i == 0),
                    stop=(i == NT - 1),
                )
                nc.tensor.matmul(
                    acc_slice2(8, K_FAC),
                    lw2[:rows, :],
                    Tbi[:rows, :],
                    start=(i == 0),
                    stop=(i == NT - 1),
                )
                rec = spool.tile([128, 1], F32, tag="rec")
                nc.vector.reciprocal(rec[:rows, :], dv[:rows, :])
                nc.scalar.sqrt(degv_all[:rows, i : i + 1], rec[:rows, :])

            # ---- int columns of H_out, written once, after every H write ----
            # Odd rows share a 64-byte HBM line with the H-part write of the
            # previous row, so the int write must not overlap any H write in
            # time.  The guard read overlaps every tile's H region (RAW: it
            # waits for all H writes) and the int region (WAR: the int write
            # waits for it).
            guard = cpool.tile([NT, 16], F32)
            nc.sync.dma_start(guard[:], hout_d[64:SHARD:128, 0:16])
            full = (NT - 1) * 128
            nc.sync.dma_start(
                hout_d[:full, 0:K_FAC].rearrange("(i p) k -> p i k", p=128),
                intall[:, : NT - 1, :],
            )
            nc.sync.dma_start(
                hout_d[full:, 0:K_FAC], intall[: SHARD - full, NT - 1, :]
            )

            # degV out: degv_all[p, i] -> degV[i*128 + p]
            nc.scalar.dma_start(
                degv_d[:full, :].rearrange("(i p) one -> p (i one)", p=128),
                degv_all[:, : NT - 1],
            )
            nc.scalar.dma_start(
                degv_d[full:, :], degv_all[: SHARD - full, NT - 1 : NT]
            )

            # ---------------- phase C: AllReduce partials, compute degE --------
            # acc_sb rows: cnt -> c, dA -> 9+c, dB -> 18+c, dC -> 27+c,
            # dD -> 36+c
            acc_sb = cpool.tile([54, 512], F32)
            nc.vector.memset(acc_sb[0:54, :], 0.0)
            for c in range(9):
                w = 512 if c < 8 else K_FAC
                off = 32 * (c % 3)
                mir = spool.tile([128, 512], F32, tag="mir")
                nc.vector.tensor_copy(mir[off : off + 3, :w], acc_slice(c, w))
                mir2 = spool.tile([128, 512], F32, tag="mir2")
                nc.vector.tensor_copy(mir2[off : off + 3, :w], acc_slice2(c, w))
                for j in range(3):
                    nc.sync.dma_start(
                        acc_sb[9 * j + c : 9 * j + c + 1, :w],
                        mir[off + j : off + j + 1, :w],
                    )
                for j in range(3):
                    nc.sync.dma_start(
                        acc_sb[27 + 9 * j + c : 27 + 9 * j + c + 1, :w],
                        mir2[off + j : off + j + 1, :w],
                    )
            ar_in = dpool.tile([54, 512], F32)
            nc.gpsimd.dma_start(ar_in[:], acc_sb[0:54, :])
            ar_out = dpool.tile([54, 512], F32, addr_space="Shared")
            nc.gpsimd.collective_compute(
                "AllReduce",
                ALU.add,
                replica_groups=[list(range(NC))],
                ins=[ar_in[:].opt()],
                outs=[ar_out[:].opt()],
            )
            cnt_t = cpool.tile([9, 512], F32)
            nc.gpsimd.dma_start(cnt_t[0:9, :], ar_out[0:9, :])
            hi_t = cpool.tile([9, 512], F32)
            nc.gpsimd.dma_start(hi_t[0:9, :], ar_out[9:18, :])
            lo_t = cpool.tile([9, 512], F32)
            nc.gpsimd.dma_start(lo_t[0:9, :], ar_out[18:27, :])
            sc_t = cpool.tile([9, 512], F32)
            nc.gpsimd.dma_start(sc_t[0:9, :], ar_out[27:36, :])
            sd_t = cpool.tile([9, 512], F32)
            nc.gpsimd.dma_start(sd_t[0:9, :], ar_out[36:45, :])
            se_t = cpool.tile([9, 512], F32)
            nc.gpsimd.dma_start(se_t[0:9, :], ar_out[45:54, :])

            # wsum = 4096*dA + 256*dB + 16*dC + dD via Horner; degE =
            # sqrt(max(cnt,1) / wsum).  The int chunk carries 2x everything;
            # the factor cancels in the ratio.
            nc.vector.scalar_tensor_tensor(
                hi_t[0:9, :], hi_t[0:9, :], 8.0, lo_t[0:9, :],
                ALU.mult, ALU.add,
            )
            nc.vector.scalar_tensor_tensor(
                hi_t[0:9, :], hi_t[0:9, :], 8.0, sc_t[0:9, :],
                ALU.mult, ALU.add,
            )
            nc.vector.scalar_tensor_tensor(
                hi_t[0:9, :], hi_t[0:9, :], 8.0, sd_t[0:9, :],
                ALU.mult, ALU.add,
            )
            nc.vector.scalar_tensor_tensor(
                hi_t[0:9, :], hi_t[0:9, :], 8.0, se_t[0:9, :],
                ALU.mult, ALU.add,
            )
            nc.vector.tensor_scalar_max(cnt_t[0:9, :], cnt_t[0:9, :], 1.0)
            # clamp: only affects the never-output padding cells of the int
            # row (wsum there is 0, and sqrt(inf) faults the scalar engine)
            nc.vector.tensor_scalar_max(hi_t[0:9, :], hi_t[0:9, :], 1e-30)
            nc.vector.reciprocal(hi_t[0:9, :], hi_t[0:9, :])
            rr = cpool.tile([9, 512], F32)
            nc.vector.tensor_tensor(rr[0:9, :], cnt_t[0:9, :], hi_t[0:9, :],
                                    ALU.mult)
            dege_all = cpool.tile([9, 512], F32)
            nc.scalar.sqrt(dege_all[0:9, :], rr[0:9, :])
            nc.scalar.dma_start(
                dege_d[K_FAC:, :].rearrange("(c k) one -> c (k one)", c=8),
                dege_all[0:8, :],
            )
            nc.scalar.dma_start(
                dege_d[0:K_FAC, :].rearrange("k one -> one k"),
                dege_all[8:9, 0:K_FAC],
            )

    nc.finalize()
    return nc


def _constants():
    ident = np.eye(128, dtype=np.float32)
    p = np.arange(128)
    gmat = ((p[:, None] // 2) % 8 == (p[None, :] // 2) % 8).astype(np.float32)
    return ident, gmat


def kernel(hidden, H, int_emb):
    global _CACHED
    if _CACHED is None:
        _CACHED = _build()
    nc = _CACHED
    hidden = np.ascontiguousarray(hidden, dtype=np.float32)
    H = np.ascontiguousarray(H, dtype=np.float32)
    int_emb = np.ascontiguousarray(int_emb, dtype=np.float32)
    ident, gmat = _constants()
    in_maps = [
        {
            "hidden": hidden[r * SHARD : (r + 1) * SHARD],
            "H": H[r * SHARD : (r + 1) * SHARD],
            "int_emb": int_emb,
            "ident": ident,
            "G": gmat,
        }
        for r in range(NC)
    ]
    res = bass_utils.run_bass_kernel_spmd(
        nc, in_maps, core_ids=list(range(NC)), trace=False
    )
    outs = res.results
    H_out = np.concatenate([outs[r]["Hout"] for r in range(NC)], axis=0)
    degV = np.concatenate([outs[r]["degV"] for r in range(NC)], axis=0)
    degE = outs[0]["degE"]
    return H_out, degV, degE
